# revision 52
# baseline (speedup 1.0000x reference)
"""Trainium2 Bass kernel for CrossViewDeformableBlock (sparse deformable attention).

Contract: kernel(**inputs) -> np.ndarray takes FULL inputs (as from
setup_inputs()) and returns the FULL output [b, 128, 64, 64].

Sharding: 8 cores, q-parallel. Core c handles batch b_c = c//4 and query
range [(c%4)*1024, +1024) of the 64*64=4096 BEV queries. Each core builds
bf16 K|V image tables for its 6 cameras on-device, computes projection /
offsets / bilinear sample coordinates on-device, gathers paired 2-position
rows with dma_gather, bilinear-blends corners (split across the DVE and
Act engines), does the point-softmax attention and output projection, and
writes its [1024, 128] output shard. The host only slices inputs,
transposes weights, and casts the image features / kv weights to bf16
(layout/dtype staging).

Algebraic simplifications vs the reference:
  - the K bias (bkv[:128]) shifts sim by a constant per (q, head) and
    cancels in the softmax -> dropped entirely;
  - the V bias (bkv[128:]) is a constant output offset (attention weights
    sum to 1) -> folded into the output-projection bias via one matmul;
  - pinhole projection composed as a single 3x4 matrix MT = (K E[:3])^T
    on PE, applied per q-tile with one matmul for all 6 cams.

Emission order (chosen so engine queues never head-of-line block):
  1. offsets + projection matmuls, batched per-cam DVE coordinate math;
  2. kv tables for cams 0/1: one whole-image DMA, 22 matmuls into an SBUF
     stage, 3 large pattern DMAs out (paired-row interleave in the dram
     access pattern); tables 2-5 are built the same way, spread across
     the attention iterations two cameras ahead of their use;
  3. gather index tables: partition rewrap via 8 SBUF->SBUF DMAs plus a
     replication matmul reading a permuted access pattern;
  4. attention loop, cam-outer / q-tile-inner: dma_gather of 2KB paired
     corner rows -> x/y lerp blends (LERP2 custom DVE op for some points,
     Act-scale pairs + DVE add for others) -> K.q dot + point softmax
     (exp on Act, 1/NCAM folded into the Act-side att expansion that also
     makes the V-weight multiply a fast-mode stride-1 op) -> V sum;
  5. per-q-tile output projection emitted as soon as its last cam is done.
"""

import math
import os
import numpy as np

import concourse.bass as bass
import concourse.mybir as mybir
import concourse.tile as tile
from concourse import bacc
from concourse.bass import ts
from concourse.masks import make_identity

# ---------------------------------------------------------------- constants
B, NCAM, H, W = 2, 6, 64, 64
HW = H * W                      # 4096 queries per batch
IH, IW = 32, 88                 # image feature h, w
IHW = IH * IW                   # 2816 positions
HEADS, DH, INNER = 4, 32, 128
NP = 8                          # sample points per query
DIM = 128
NCORES = 8
QPC = HW // (NCORES // B)       # 1024 queries per core
NQT = QPC // 128                # 8 q-tiles of 128
PADROWS = 2944                  # 23 * 128 rows in kv table (2816 + 128 pad)
KVROW = 2 * INNER               # 256 channels (K|V) per position
F32 = mybir.dt.float32
BF16 = mybir.dt.bfloat16
I16 = mybir.dt.int16


def _register_lerp_op():
    """Register LERP2: out = in0*s0 + in1*s1 (per-partition scalars s0,s1)."""
    from concourse.dve_spec import Spec, Src0, Src1, C0, C1, lower
    from concourse.dve_spec import _has_src1 as has_src1
    from concourse.dve_uop import DveOpSpec
    from concourse.dve_ops import DveOp, OPS, _SUB_OPCODE_FOR_NAME, _CUSTOM_DVE_ROW_BASE

    if "LERP2" in _SUB_OPCODE_FOR_NAME:
        for op in OPS:
            if op.name == "LERP2":
                return op
    spec = Spec(
        body=Src0 * C0 + Src1 * C1,
        reference=lambda in0, in1, s0, s1, imm2: (
            in0.astype(np.float32) * s0 + in1.astype(np.float32) * s1
        ),
    )
    opcode = _CUSTOM_DVE_ROW_BASE + len(OPS)
    assert opcode < 0x20
    shas = {}
    for ver in ("v3", "v4"):
        try:
            r = DveOpSpec(name="LERP2", opcode=opcode, uops=lower(spec, ver=ver),
                          rd1_en=has_src1(spec))
            shas[ver] = r.sha(ver)
        except Exception:
            pass
    op = DveOp("LERP2", spec, subdim=False, uops_sha=shas,
               perf_en={v: True for v in shas})
    OPS.append(op)
    _SUB_OPCODE_FOR_NAME["LERP2"] = opcode
    from concourse import dve_ops as _do
    _do.CUSTOM_DVE_SPECS["LERP2"] = spec
    return op


def build_kernel(nc):
    """Emit the SPMD program. All per-core variation comes via input data."""
    lerp_op = _register_lerp_op()

    # ---------------- dram I/O ----------------
    img = nc.dram_tensor("img", [NCAM, DIM, IHW], BF16, kind="ExternalInput").ap()
    wkvT = nc.dram_tensor("wkvT", [DIM, KVROW], BF16, kind="ExternalInput").ap()
    bv_c = nc.dram_tensor("bv_c", [INNER, 1], F32, kind="ExternalInput").ap()
    bev_l = nc.dram_tensor("bev_l", [DIM, QPC], F32, kind="ExternalInput").ap()
    wxy_l = nc.dram_tensor("wxy_l", [2, QPC], F32, kind="ExternalInput").ap()
    E_l = nc.dram_tensor("E_l", [4, NCAM * 4], F32, kind="ExternalInput").ap()
    KT = nc.dram_tensor("KT", [3, NCAM * 3], F32, kind="ExternalInput").ap()
    wqT = nc.dram_tensor("wqT", [DIM, INNER], F32, kind="ExternalInput").ap()
    bq_r = nc.dram_tensor("bq_r", [1, INNER], BF16, kind="ExternalInput").ap()
    w1T = nc.dram_tensor("w1T", [DIM, DIM], F32, kind="ExternalInput").ap()
    b1 = nc.dram_tensor("b1", [DIM, 1], F32, kind="ExternalInput").ap()
    w2T = nc.dram_tensor("w2T", [DIM, 2 * NP], F32, kind="ExternalInput").ap()
    b2 = nc.dram_tensor("b2", [2 * NP, 1], F32, kind="ExternalInput").ap()
    wpT = nc.dram_tensor("wpT", [INNER, DIM], F32, kind="ExternalInput").ap()
    bp_r = nc.dram_tensor("bp_r", [1, DIM], BF16, kind="ExternalInput").ap()
    cst01 = nc.dram_tensor("cst01", [2, QPC], F32, kind="ExternalInput").ap()
    rep_in = nc.dram_tensor("rep_in", [16, 128], F32, kind="ExternalInput").ap()
    out_l = nc.dram_tensor("out_l", [QPC, DIM], F32, kind="ExternalOutput").ap()

    with tile.TileContext(nc) as tc:
        _emit(tc, nc, lerp_op, img, wkvT, bv_c, bev_l, wxy_l, E_l, KT, wqT,
              bq_r, w1T, b1, w2T, b2, wpT, bp_r, cst01, rep_in, out_l)
    return nc


def _lerp(nc, lerp_op, out, in0, in1, s0, s1):
    """out = in0*s0 + in1*s1 with s0/s1 [P,1] columns."""
    nc.vector._custom_dve(lerp_op, out=out, in0=in0, in1=in1, s0=s0, s1=s1)


def _emit(tc, nc, lerp_op, img, wkvT, bv_c, bev_l, wxy_l, E_l, KT, wqT,
          bq_r, w1T, b1, w2T, b2, wpT, bp_r, cst01, rep_in, out_l):
    import contextlib
    ctx = contextlib.ExitStack()
    with ctx:
        singles = ctx.enter_context(tc.tile_pool(name="singles", bufs=1))
        cpool = ctx.enter_context(tc.tile_pool(name="cpool", bufs=2))
        temps = ctx.enter_context(tc.tile_pool(name="temps", bufs=3))
        stpool = ctx.enter_context(tc.tile_pool(name="stpool", bufs=2))
        gath = ctx.enter_context(tc.tile_pool(name="gath", bufs=4))
        blend = ctx.enter_context(tc.tile_pool(name="blend", bufs=2))
        stats = ctx.enter_context(tc.tile_pool(name="stats", bufs=4))
        apool = ctx.enter_context(tc.tile_pool(name="apool", bufs=3))
        psum = ctx.enter_context(tc.tile_pool(name="psum", bufs=3, space="PSUM"))
        psum2 = ctx.enter_context(tc.tile_pool(name="psum2", bufs=2, space="PSUM"))
        dram = ctx.enter_context(tc.tile_pool(name="dram", bufs=1, space="DRAM"))

        AX = mybir.AxisListType
        ALU = mybir.AluOpType
        ACTF = mybir.ActivationFunctionType

        # ------------- resident tiles -------------
        ident = singles.tile([128, 128], F32)
        make_identity(nc, ident[:])
        wkvT_sb = singles.tile([DIM, KVROW], BF16)
        nc.sync.dma_start(out=wkvT_sb[:], in_=wkvT)
        bvc_sb = singles.tile([INNER, 1], F32)
        nc.sync.dma_start(out=bvc_sb[:], in_=bv_c)
        bev_sb = singles.tile([DIM, QPC], F32)
        nc.sync.dma_start(out=bev_sb[:], in_=bev_l)
        wqT_sb = singles.tile([DIM, INNER], F32)
        nc.sync.dma_start(out=wqT_sb[:], in_=wqT)
        bq_sb = singles.tile([1, INNER], BF16)
        nc.sync.dma_start(out=bq_sb[:], in_=bq_r)
        w1T_sb = singles.tile([DIM, DIM], F32)
        nc.sync.dma_start(out=w1T_sb[:], in_=w1T)
        w2T_sb = singles.tile([DIM, 2 * NP], F32)
        nc.sync.dma_start(out=w2T_sb[:], in_=w2T)
        wpT_sb = singles.tile([INNER, DIM], F32)
        nc.sync.dma_start(out=wpT_sb[:], in_=wpT)
        bp_sb = singles.tile([1, DIM], BF16)
        nc.sync.dma_start(out=bp_sb[:], in_=bp_r)
        b1_sb = singles.tile([DIM, 1], F32)
        nc.sync.dma_start(out=b1_sb[:], in_=b1)
        b2_sb = singles.tile([2 * NP, 1], F32)
        nc.sync.dma_start(out=b2_sb[:], in_=b2)
        E_sb = singles.tile([4, NCAM * 4], F32)
        nc.sync.dma_start(out=E_sb[:], in_=E_l)
        KT_sb = singles.tile([3, NCAM * 3], F32)
        nc.sync.dma_start(out=KT_sb[:], in_=KT)
        REP_sb = singles.tile([16, 128], F32)
        nc.sync.dma_start(out=REP_sb[:], in_=rep_in)
        ones_bf = singles.tile([1, 128], BF16)
        nc.vector.memset(ones_bf[:], 1.0)

        # xyz1 = [wx, wy, 0, 1]
        xyz1_sb = singles.tile([128, QPC], F32)
        nc.sync.dma_start(out=xyz1_sb[0:2, :], in_=wxy_l)
        nc.sync.dma_start(out=xyz1_sb[2:4, :], in_=cst01)

        # resident products of phase A
        qbf_sb = singles.tile([128, QPC], BF16)
        off_t_all = singles.tile([128, NQT * 16], F32)
        wA_sb = singles.tile([128, NCAM * NQT * 16], F32)
        wB_sb = singles.tile([128, NCAM * NQT * 16], F32)
        idx2_all = singles.tile([128, NCAM * NQT * NP], F32)
        # xyz1's rows are startup-only; its byte range doubles as the
        # loop-only wacc accumulator (frees 4KB of SBUF)
        wacc_all = xyz1_sb
        T_tiles = [singles.tile([128, NCAM * 64], I16, tag=f"Tq{qt}",
                                name=f"Tq{qt}")
                   for qt in range(NQT)]

        # per-cam kv tables in DRAM; row y*IW+x holds KV(y,x) ++ KV(y+1,x)
        kv_cam = [dram.tile([PADROWS, 2 * KVROW], BF16, tag=f"kv{c}",
                            name=f"kv{c}")
                  for c in range(NCAM)]

        # ---------------- kv table builder ----------------
        zt = singles.tile([128, KVROW], BF16)
        nc.vector.memset(zt[:], 0)
        NPT = IHW // 128  # 22 position tiles

        # zero pads for every cam upfront (no deps beyond the zt memset)
        for _c in range(NCAM):
            kd = kv_cam[_c]
            nc.sync.dma_start(out=kd[IHW:PADROWS, 0:KVROW], in_=zt[:])
            nc.sync.dma_start(out=kd[IHW - IW:IHW - IW + 128, KVROW:2 * KVROW],
                              in_=zt[:])
            nc.sync.dma_start(out=kd[IHW - IW + 128:PADROWS, KVROW:2 * KVROW],
                              in_=zt[:PADROWS - (IHW - IW + 128), :])

        # table build: one whole-image DMA in, 22 matmul+copy into an SBUF
        # stage, then 3 large interleaved DMAs out (the paired-row layout's
        # (pt, p) -> row interleave is expressed in the dram access pattern)
        build_state = {}
        ROWE = 2 * KVROW  # 512 elements per table row

        def build_start(cam):
            img_sb = stpool.tile([128, IHW], BF16, tag="imgsb", name="imgsb")
            nc.sync.dma_start(out=img_sb[:], in_=img[cam])
            stage = stpool.tile([128, NPT * KVROW], BF16, tag="stage",
                                name="stage")
            build_state[cam] = (img_sb, stage)

        def build_chunk(cam, lo, hi):
            # two position-tiles share one psum bank and one Act copy:
            # halves the Act-engine copy count during the loop phase where
            # Act is the tighter engine
            img_sb, stage = build_state[cam]
            pts = list(range(lo, min(hi, NPT)))
            for i in range(0, len(pts), 2):
                pr = pts[i:i + 2]
                kv_ps = psum.tile([128, 2 * KVROW], F32, tag="mm2")
                for j, pt in enumerate(pr):
                    nc.tensor.matmul(out=kv_ps[:, ts(j, KVROW)],
                                     lhsT=img_sb[:, ts(pt, 128)],
                                     rhs=wkvT_sb[:], start=True, stop=True)
                nc.scalar.activation(
                    out=stage[:, pr[0] * KVROW:(pr[0] + len(pr)) * KVROW],
                    in_=kv_ps[:, 0:len(pr) * KVROW], func=ACTF.Copy)

        def build_store(cam):
            img_sb, stage = build_state.pop(cam)
            kd = kv_cam[cam]
            kd0 = kd[:].offset
            sap = stage[:]
            # first half: table row pt*128+p, col 0:256
            out1 = bass.AP(tensor=kd.tensor, offset=kd0,
                           ap=[[ROWE, 128], [128 * ROWE, NPT], [1, KVROW]])
            in1 = bass.AP(tensor=stage.tensor, offset=sap.offset,
                          ap=[sap.ap[0], [KVROW, NPT], [1, KVROW]])
            nc.sync.dma_start(out=out1, in_=in1)
            # second half (shifted by IW): rows 0..127-IW from tile 0
            out2 = bass.AP(tensor=kd.tensor, offset=kd0 + KVROW,
                           ap=[[ROWE, 128 - IW], [1, KVROW]])
            s2 = stage[IW:128, :]
            in2 = bass.AP(tensor=stage.tensor, offset=s2.offset,
                          ap=[s2.ap[0], [1, KVROW]])
            nc.sync.dma_start(out=out2, in_=in2)
            # second half rows (128-IW).. from tiles 1..21
            out3 = bass.AP(tensor=kd.tensor,
                           offset=kd0 + (128 - IW) * ROWE + KVROW,
                           ap=[[ROWE, 128], [128 * ROWE, NPT - 1], [1, KVROW]])
            in3 = bass.AP(tensor=stage.tensor, offset=sap.offset + KVROW,
                          ap=[sap.ap[0], [KVROW, NPT - 1], [1, KVROW]])
            nc.sync.dma_start(out=out3, in_=in3)

        # cam 0/1 tables first: their matmuls/copies/DMAs have no deps on
        # phase A, so PE/Act/DMA stream them at full rate while nothing else
        # is ready; phase A's engine ping-pong then overlaps the store DMAs
        # offsets: o1 = relu(w1 @ bev + b1); off = w2 @ o1 + b2  [16, QPC]
        o1_sb = singles.tile([DIM, QPC], F32)
        for hf in range(2):
            o1_ps = psum2.tile([DIM, QPC // 2], F32, tag="wide")
            nc.tensor.matmul(out=o1_ps[:], lhsT=w1T_sb[:],
                             rhs=bev_sb[:, ts(hf, QPC // 2)], start=True, stop=True)
            nc.scalar.activation(out=o1_sb[:, ts(hf, QPC // 2)], in_=o1_ps[:],
                                 func=ACTF.Relu, bias=b1_sb[:], scale=1.0)
        off_sb = singles.tile([2 * NP, QPC], F32)  # rows: c*8+p (x offs 0-7, y offs 8-15)
        for hf in range(2):
            off_ps = psum2.tile([2 * NP, QPC // 2], F32, tag="wide")
            nc.tensor.matmul(out=off_ps[:], lhsT=w2T_sb[:],
                             rhs=o1_sb[:, ts(hf, QPC // 2)], start=True, stop=True)
            nc.scalar.activation(out=off_sb[:, ts(hf, QPC // 2)], in_=off_ps[:],
                                 func=ACTF.Identity, bias=b2_sb[:], scale=1.0)

        # transpose offsets once per q-tile: off_t_all [128, (qt, 16)]
        for qt in range(NQT):
            ot_ps = psum.tile([128, 2 * NP], F32, tag="mm")
            nc.tensor.transpose(out=ot_ps[:], in_=off_sb[:, ts(qt, 128)],
                                identity=ident[:2 * NP, :2 * NP])
            nc.scalar.activation(out=off_t_all[:, ts(qt, 2 * NP)], in_=ot_ps[:],
                                 func=ACTF.Copy)

        # ---------------- A: per-cam projection + coords ----------------
        BIGF = 8388608.0
        halfx = 0.5 * (IW - 1)
        halfy = 0.5 * (IH - 1)
        # MT = (K @ E[:3,:])^T [4,3] per cam, computed directly:
        # MT[j,i] = sum_k E[k,j] K[i,k] = matmul(lhsT=E_rows, rhs=K^T)
        MT_sb = singles.tile([4, NCAM * 3], F32)
        for cam in range(NCAM):
            mt_ps = psum.tile([4, 3], F32, tag="mm")
            nc.tensor.matmul(out=mt_ps[:], lhsT=E_sb[0:3, ts(cam, 4)],
                             rhs=KT_sb[:, ts(cam, 3)], start=True, stop=True)
            nc.scalar.activation(out=MT_sb[:, ts(cam, 3)], in_=mt_ps[:],
                                 func=ACTF.Copy)

        # pxt_all [128, (qt, cam, 3)]: all cams' projections in one matmul/qt
        pxt_all = singles.tile([128, NQT * NCAM * 3], F32)
        for qt in range(NQT):
            pt_ps = psum.tile([128, NCAM * 3], F32, tag="mm")
            nc.tensor.matmul(out=pt_ps[:], lhsT=xyz1_sb[0:4, ts(qt, 128)],
                             rhs=MT_sb[:], start=True, stop=True)
            nc.scalar.activation(out=pxt_all[:, ts(qt, NCAM * 3)], in_=pt_ps[:],
                                 func=ACTF.Copy)

        pap = pxt_all[:]
        oap = off_t_all[:]
        for cj in range(NCAM // 2):
            c0 = 2 * cj  # cams c0, c0+1 batched together

            def _px(col):  # [128, (2cam, qt)] slice of pxt_all (qt, cam, 3)
                return bass.AP(tensor=pxt_all.tensor,
                               offset=pap.offset + c0 * 3 + col,
                               ap=[pap.ap[0], [3, 2], [NCAM * 3, NQT]])

            # rden = 1 / max(pz, 1e-6)
            rden = cpool.tile([128, 2, NQT], F32, tag="rden")
            nc.vector.tensor_scalar(out=rden[:], in0=_px(2), scalar1=1e-6,
                                    scalar2=None, op0=ALU.max)
            nc.vector.reciprocal(out=rden[:], in_=rden[:])
            # g = uv/(dim-1)*2 - 1
            gx = cpool.tile([128, 2, NQT], F32, tag="gx")
            nc.vector.tensor_tensor(out=gx[:], in0=_px(0), in1=rden[:],
                                    op=ALU.mult)
            nc.vector.tensor_scalar(out=gx[:], in0=gx[:],
                                    scalar1=2.0 / (IW - 1), scalar2=1.0,
                                    op0=ALU.mult, op1=ALU.subtract)
            gy = cpool.tile([128, 2, NQT], F32, tag="gy")
            nc.vector.tensor_tensor(out=gy[:], in0=_px(1), in1=rden[:],
                                    op=ALU.mult)
            nc.vector.tensor_scalar(out=gy[:], in0=gy[:],
                                    scalar1=2.0 / (IH - 1), scalar2=1.0,
                                    op0=ALU.mult, op1=ALU.subtract)

            # sxy [128, (cam2, qt, 16)]: samp = clip(off+g, -1, 1) -> pixels
            sxy = cpool.tile([128, 2 * NQT * 16], F32, tag="sxy")
            sap = sxy[:]

            def _sl(t, tap, off0):  # [128, (cam2, qt, 8)] x(0)/y(8) slices
                return bass.AP(tensor=t.tensor, offset=tap.offset + off0,
                               ap=[tap.ap[0], [NQT * 16, 2], [16, NQT],
                                   [1, NP]])

            def _obc(off0):  # off_t_all bc over the 2 cams
                return bass.AP(tensor=off_t_all.tensor,
                               offset=oap.offset + off0,
                               ap=[oap.ap[0], [0, 2], [16, NQT], [1, NP]])

            def _gbc(g):  # gx/gy bc over p
                gp = g[:]
                return bass.AP(tensor=g.tensor, offset=gp.offset,
                               ap=[gp.ap[0], [NQT, 2], [1, NQT], [0, NP]])

            nc.vector.tensor_tensor(out=_sl(sxy, sap, 0), in0=_obc(0),
                                    in1=_gbc(gx), op=ALU.add)
            nc.vector.tensor_tensor(out=_sl(sxy, sap, NP), in0=_obc(NP),
                                    in1=_gbc(gy), op=ALU.add)
            nc.vector.tensor_scalar(out=sxy[:], in0=sxy[:], scalar1=1.0,
                                    scalar2=-1.0, op0=ALU.min, op1=ALU.max)
            nc.vector.tensor_scalar(out=_sl(sxy, sap, 0), in0=_sl(sxy, sap, 0),
                                    scalar1=1.0, scalar2=halfx,
                                    op0=ALU.add, op1=ALU.mult)
            nc.vector.tensor_scalar(out=_sl(sxy, sap, NP), in0=_sl(sxy, sap, NP),
                                    scalar1=1.0, scalar2=halfy,
                                    op0=ALU.add, op1=ALU.mult)

            # floor via +2^23 round-to-nearest, then fixup so frac >= 0
            rnd = cpool.tile([128, 2 * NQT * 16], F32, tag="rnd")
            nc.vector.tensor_scalar(out=rnd[:], in0=sxy[:], scalar1=BIGF,
                                    scalar2=BIGF, op0=ALU.add, op1=ALU.subtract)
            dfr = cpool.tile([128, 2 * NQT * 16], F32, tag="dfr")
            nc.vector.tensor_tensor(out=dfr[:], in0=sxy[:], in1=rnd[:],
                                    op=ALU.subtract)
            msk = cpool.tile([128, 2 * NQT * 16], F32, tag="msk")
            nc.vector.tensor_scalar(out=msk[:], in0=dfr[:], scalar1=0.0,
                                    scalar2=None, op0=ALU.is_lt)
            x0y0 = sxy  # sxy is dead after dfr; reuse its buffer
            nc.vector.tensor_tensor(out=x0y0[:], in0=rnd[:], in1=msk[:],
                                    op=ALU.subtract)
            camw = c0 * NQT * 16
            nc.vector.tensor_tensor(out=wB_sb[:, camw:camw + 2 * NQT * 16],
                                    in0=dfr[:], in1=msk[:], op=ALU.add)
            nc.vector.tensor_scalar(out=wA_sb[:, camw:camw + 2 * NQT * 16],
                                    in0=wB_sb[:, camw:camw + 2 * NQT * 16],
                                    scalar1=-1.0, scalar2=1.0,
                                    op0=ALU.mult, op1=ALU.add)
            # idx = y0*IW + x0 (local per cam); idx2_all layout (qt, cam, p)
            xap = x0y0[:]
            rap = rnd[:]  # rnd is dead after x0y0; reuse its buffer for tmp
            tmp = bass.AP(tensor=rnd.tensor, offset=rap.offset,
                          ap=[rap.ap[0], [64, 2], [NP, NQT], [1, NP]])
            nc.vector.tensor_scalar(out=tmp, in0=_sl(x0y0, xap, NP),
                                    scalar1=float(IW), scalar2=None,
                                    op0=ALU.mult)
            i2 = idx2_all[:]
            idst = bass.AP(tensor=idx2_all.tensor,
                           offset=i2.offset + c0 * NP,
                           ap=[i2.ap[0], [NP, 2], [NCAM * NP, NQT], [1, NP]])
            nc.vector.tensor_tensor(out=idst, in0=tmp,
                                    in1=_sl(x0y0, xap, 0), op=ALU.add)

        build_start(0)
        build_chunk(0, 0, NPT)
        build_store(0)
        build_start(1)
        build_chunk(1, 0, NPT)
        build_store(1)



        # ---------------- A: queries ----------------
        for qt in range(NQT):
            q_ps = psum.tile([128, INNER], F32, tag="mm")
            nc.tensor.matmul(out=q_ps[:], lhsT=ones_bf[:], rhs=bq_sb[:],
                             start=True, stop=False)
            nc.tensor.matmul(out=q_ps[:], lhsT=bev_sb[:, ts(qt, 128)],
                             rhs=wqT_sb[:], start=False, stop=True)
            nc.scalar.activation(out=qbf_sb[:, ts(qt, INNER)], in_=q_ps[:],
                                 func=ACTF.Copy)

        # fold the V bias through the output projection: bpp = bp + bv @ wpT
        # (bk cancels in the softmax; bv is a constant output offset since the
        # attention weights sum to 1)
        bvp_ps = psum.tile([1, DIM], F32, tag="mm")
        nc.tensor.matmul(out=bvp_ps[:], lhsT=bvc_sb[:], rhs=wpT_sb[:],
                         start=True, stop=True)
        bpp_sb = singles.tile([1, DIM], BF16)
        nc.vector.tensor_tensor(out=bpp_sb[:], in0=bvp_ps[:], in1=bp_sb[:],
                                op=ALU.add)

        # ---------------- B: gather index tables ----------------
        # Need T[16k+pl, cam*64 + p*8 + qh] = idx2_all[qh*16+pl, (qt, cam, p)].
        # 8 contiguous SBUF->SBUF DMAs rewrap partitions into a [16, (qh,cam,p)]
        # scratch; the replication matmul reads it through a (cam,p,qh)-permuted
        # access pattern, producing the gather's required column order.
        tsc = singles.tile([16, 8 * NQT * NCAM * NP], F32)
        for qh in range(8):
            nc.sync.dma_start(
                out=tsc[:, qh * 384:(qh + 1) * 384],
                in_=idx2_all[qh * 16:(qh + 1) * 16, :])
        tap = tsc[:]
        for qt in range(NQT):
            rhs_perm = bass.AP(tensor=tsc.tensor, offset=tap.offset + qt * 48,
                               ap=[tap.ap[0], [NP, NCAM], [1, NP], [384, 8]])
            rep_ps = psum2.tile([128, NCAM * 64], F32, tag="wide")
            nc.tensor.matmul(out=rep_ps[:], lhsT=REP_sb[:],
                             rhs=rhs_perm, start=True, stop=True)
            nc.vector.tensor_copy(out=T_tiles[qt][:], in_=rep_ps[:])

        # ---------------- C/D/E: attention, cam-outer ----------------
        # cam c+1's kv table tiles are emitted spread across cam c's q-tile
        # iterations so their psum->sbuf copies never head-of-line-block the
        # Act engine ahead of the softmax exp.
        XOFF, YOFF = 5, 3   # points whose blend mults run on the Act engine

        def _alerp(out, in0, in1, sa, sb, n):
            # ta/tb in a deeper dedicated pool so the Act mults can run
            # further ahead of the DVE combine-adds
            tA = apool.tile([128, n], BF16, tag=f"ta{n}")
            nc.scalar.activation(out=tA[:], in_=in0, func=ACTF.Copy, scale=sa)
            tB = apool.tile([128, n], BF16, tag=f"tb{n}")
            nc.scalar.activation(out=tB[:], in_=in1, func=ACTF.Copy, scale=sb)
            nc.vector.tensor_tensor(out=out, in0=tA[:], in1=tB[:], op=ALU.add)

        for cam in range(NCAM):
            camv = kv_cam[cam]
            kv_view = bass.AP(tensor=camv.tensor, offset=camv[:].offset,
                              ap=[[2 * KVROW, PADROWS - 1], [1, 4 * KVROW]])
            for qt in range(NQT):
                if cam + 2 < NCAM:
                    # table c+2 spread over cam c's iterations: it is complete
                    # one full camera before its gathers start, so the gather
                    # prefetch never stalls on a table write
                    if qt == 0:
                        build_start(cam + 2)
                    build_chunk(cam + 2, qt * 3, qt * 3 + 3)
                    if qt == NQT - 1:
                        build_store(cam + 2)
                kvraw = gath.tile([128, NP, 4 * KVROW], BF16, tag="kvraw")
                nc.gpsimd.dma_gather(
                    out_ap=kvraw[:], in_ap=kv_view,
                    idxs_ap=T_tiles[qt][:, ts(cam, 64)],
                    num_idxs=1024, num_idxs_reg=1024,
                    elem_size=4 * KVROW, elem_step=2 * KVROW,
                    single_packet=False)
                # x-blend: 8 rows of 512 [(y0,y1) x (K|V)]
                blkw = (cam * NQT + qt) * 16
                kvx = blend.tile([128, NP, 2 * KVROW], BF16, tag="kvx")
                for p in range(NP):
                    sa = wA_sb[:, blkw + p:blkw + p + 1]
                    sb = wB_sb[:, blkw + p:blkw + p + 1]
                    if p < XOFF:
                        _alerp(kvx[:, p, :], kvraw[:, p, 0:2 * KVROW],
                               kvraw[:, p, 2 * KVROW:4 * KVROW], sa, sb,
                               2 * KVROW)
                    else:
                        _lerp(nc, lerp_op, kvx[:, p, :],
                              kvraw[:, p, 0:2 * KVROW],
                              kvraw[:, p, 2 * KVROW:4 * KVROW], sa, sb)
                # y-blend: 8 points of 256
                kvb = blend.tile([128, NP, KVROW], BF16, tag="kvb")
                for p in range(NP):
                    sa = wA_sb[:, blkw + 8 + p:blkw + 9 + p]
                    sb = wB_sb[:, blkw + 8 + p:blkw + 9 + p]
                    if p < YOFF:
                        _alerp(kvb[:, p, :], kvx[:, p, 0:KVROW],
                               kvx[:, p, KVROW:2 * KVROW], sa, sb, KVROW)
                    else:
                        _lerp(nc, lerp_op, kvb[:, p, :],
                              kvx[:, p, 0:KVROW], kvx[:, p, KVROW:2 * KVROW],
                              sa, sb)
                # K dot q -> sim [128, p, h]
                up = blend.tile([128, NP, INNER], BF16, tag="up")
                qv = qbf_sb[:, ts(qt, INNER)]
                nc.vector.tensor_tensor(
                    out=up[:], in0=kvb[:, :, 0:INNER],
                    in1=bass.AP(tensor=qbf_sb.tensor, offset=qv.offset,
                                ap=[qv.ap[0], [0, NP], [1, INNER]]),
                    op=ALU.mult)
                sim = stats.tile([128, NP, HEADS], F32, tag="sim")
                upap = up[:]
                nc.vector.tensor_reduce(
                    out=sim[:],
                    in_=bass.AP(tensor=up.tensor, offset=upap.offset,
                                ap=[upap.ap[0], [INNER, NP], [DH, HEADS], [1, DH]]),
                    axis=AX.X, op=ALU.add)
                # softmax over p (mean over cams folded into att expansion)
                esim = stats.tile([128, NP, HEADS], BF16, tag="esim")
                nc.scalar.activation(out=esim[:], in_=sim[:], func=ACTF.Exp)
                ssum = stats.tile([128, HEADS], F32, tag="ssum")
                esap = esim[:]
                nc.vector.tensor_reduce(
                    out=ssum[:],
                    in_=bass.AP(tensor=esim.tensor, offset=esap.offset,
                                ap=[esap.ap[0], [1, HEADS], [HEADS, NP]]),
                    axis=AX.X, op=ALU.add)
                srec = stats.tile([128, HEADS], F32, tag="srec")
                nc.vector.reciprocal(out=srec[:], in_=ssum[:])
                att = stats.tile([128, NP, HEADS], BF16, tag="att")
                srap = srec[:]
                nc.vector.tensor_tensor(
                    out=att[:], in0=esim[:],
                    in1=bass.AP(tensor=srec.tensor, offset=srap.offset,
                                ap=[srap.ap[0], [0, NP], [1, HEADS]]),
                    op=ALU.mult)
                # expand att over DH on the Act engine (folds the 1/NCAM mean);
                # keeps the V-weight multiply stride-1 for the DVE fast path
                atx = blend.tile([128, NP, HEADS, DH], BF16, tag="atx")
                atap = att[:]
                nc.scalar.activation(
                    out=atx[:],
                    in_=bass.AP(tensor=att.tensor, offset=atap.offset,
                                ap=[atap.ap[0], [HEADS, NP], [1, HEADS], [0, DH]]),
                    func=ACTF.Copy, scale=1.0 / NCAM)
                vw = blend.tile([128, NP, INNER], BF16, tag="vw")
                nc.vector.tensor_tensor(out=vw[:], in0=kvb[:, :, INNER:KVROW],
                                        in1=atx[:], op=ALU.mult)
                vwap = vw[:]
                vsum_in = bass.AP(tensor=vw.tensor, offset=vwap.offset,
                                  ap=[vwap.ap[0], [1, INNER], [INNER, NP]])
                if cam == 0:
                    nc.vector.tensor_reduce(out=wacc_all[:, ts(qt, INNER)],
                                            in_=vsum_in, axis=AX.X, op=ALU.add)
                else:
                    wsum = stats.tile([128, INNER], F32, tag="wsum")
                    nc.vector.tensor_reduce(out=wsum[:], in_=vsum_in,
                                            axis=AX.X, op=ALU.add)
                    nc.vector.tensor_tensor(out=wacc_all[:, ts(qt, INNER)],
                                            in0=wacc_all[:, ts(qt, INNER)],
                                            in1=wsum[:], op=ALU.add)
                if cam == NCAM - 1:
                    # output projection for this q-tile, overlapped with the
                    # remaining iterations
                    wt_ps = psum.tile([128, 128], F32, tag="mm")
                    nc.tensor.transpose(out=wt_ps[:],
                                        in_=wacc_all[:, ts(qt, INNER)],
                                        identity=ident[:])
                    waccT = temps.tile([128, 128], F32, tag="waccT")
                    nc.scalar.activation(out=waccT[:], in_=wt_ps[:],
                                         func=ACTF.Copy)
                    out_ps = psum.tile([128, DIM], F32, tag="mm")
                    nc.tensor.matmul(out=out_ps[:], lhsT=ones_bf[:],
                                     rhs=bpp_sb[:], start=True, stop=False)
                    nc.tensor.matmul(out=out_ps[:], lhsT=waccT[:],
                                     rhs=wpT_sb[:], start=False, stop=True)
                    outf = temps.tile([128, DIM], F32, tag="outf")
                    nc.scalar.activation(out=outf[:], in_=out_ps[:],
                                         func=ACTF.Copy)
                    nc.sync.dma_start(out=out_l[ts(qt, 128), :], in_=outf[:])


# ---------------------------------------------------------------- host side
_CACHED = {}


def _build():
    if "nc" not in _CACHED:
        nc = bacc.Bacc("TRN2", target_bir_lowering=False, debug=False,
                       num_devices=NCORES)
        build_kernel(nc)
        nc.compile()
        _CACHED["nc"] = nc
    return _CACHED["nc"]


def make_in_maps(inputs):
    """Slice/transpose/cast FULL inputs into 8 per-core input dicts."""
    import ml_dtypes
    BF = ml_dtypes.bfloat16
    f = lambda x: np.ascontiguousarray(np.asarray(x, dtype=np.float32))
    bev = f(inputs["bev"]).reshape(B, DIM, HW)
    img_feats = f(inputs["img_feats"]).reshape(B, NCAM, DIM, IHW)
    Kc = f(inputs["K"])
    Ec = f(inputs["E"])
    world_xy = f(inputs["world_xy"]).reshape(2, HW)
    wq = f(inputs["wq"]); bq = f(inputs["bq"])
    wkv = f(inputs["wkv"]); bkv = f(inputs["bkv"])
    w_off1 = f(inputs["w_off1"]); b_off1 = f(inputs["b_off1"])
    w_off2 = f(inputs["w_off2"]); b_off2 = f(inputs["b_off2"])
    w_proj = f(inputs["w_proj"]); b_proj = f(inputs["b_proj"])

    # row-permute w_off2/b_off2 from (p, c) to (c, p) ordering
    perm = [p * 2 + c for c in range(2) for p in range(NP)]
    w2p = w_off2[perm, :]
    b2p = b_off2[perm]

    in_maps = []
    for core in range(NCORES):
        bc = core // (NCORES // B)
        q0 = (core % (NCORES // B)) * QPC
        m = {
            "img": np.ascontiguousarray(img_feats[bc]).astype(BF),
            "wkvT": np.ascontiguousarray(wkv.T).astype(BF),
            "bv_c": bkv[INNER:].reshape(INNER, 1),
            "bev_l": np.ascontiguousarray(bev[bc, :, q0:q0 + QPC]),
            "wxy_l": np.ascontiguousarray(world_xy[:, q0:q0 + QPC]),
            "E_l": np.ascontiguousarray(Ec[bc].transpose(1, 0, 2).reshape(4, NCAM * 4)),
            "KT": np.ascontiguousarray(Kc[bc].transpose(2, 0, 1).reshape(3, NCAM * 3)),
            "wqT": np.ascontiguousarray(wq.T),
            "bq_r": bq.reshape(1, INNER).astype(BF),
            "w1T": np.ascontiguousarray(w_off1.T),
            "b1": b_off1.reshape(DIM, 1),
            "w2T": np.ascontiguousarray(w2p.T),
            "b2": b2p.reshape(2 * NP, 1),
            "wpT": np.ascontiguousarray(w_proj.T),
            "bp_r": b_proj.reshape(1, DIM).astype(BF),
            "cst01": np.concatenate([np.zeros((1, QPC), np.float32),
                                     np.ones((1, QPC), np.float32)], 0),
            "rep_in": (np.arange(128)[None, :] % 16 ==
                       np.arange(16)[:, None]).astype(np.float32),
        }
        in_maps.append(m)
    return in_maps


def assemble(results):
    """results: list of 8 dicts with out_l [QPC, DIM] -> [B, DIM, H, W]."""
    full = np.zeros((B, HW, DIM), dtype=np.float32)
    for core, r in enumerate(results):
        bc = core // (NCORES // B)
        q0 = (core % (NCORES // B)) * QPC
        full[bc, q0:q0 + QPC, :] = r["out_l"]
    return np.ascontiguousarray(full.transpose(0, 2, 1).reshape(B, DIM, H, W))


def kernel(**inputs):
    from concourse.bass_utils import run_bass_kernel_spmd
    nc = _build()
    in_maps = make_in_maps(inputs)
    res = run_bass_kernel_spmd(nc, in_maps, core_ids=list(range(NCORES)))
    return assemble(res.results)


if __name__ == "__main__":
    import reference
    inputs = {k: np.asarray(v) for k, v in reference.setup_inputs().items()}
    out = kernel(**inputs)
    exp = np.asarray(reference.reference(**{k: np.asarray(v) for k, v in inputs.items()}))
    err = np.abs(out - exp).max() / (np.abs(exp).max() + 1e-9)
    print("Relative error:", err)
